# revision 25
# baseline (speedup 1.0000x reference)
"""Trainium2 Bass kernel for a dense transformer block (B=2,S=2048,E=768,H=12,D=64,F=3072).

Sharding: 8 cores = 2 batch groups x 4 cores. Within a batch group each core
computes attention for 3 of the 12 heads over the full sequence, partial output
projections are combined with a 4-core ReduceScatter (f16 payload), and each
core then runs the FFN on its 512 rows with replicated FFN weights.

The pre-FFN section is software-pipelined over 512-token blocks processed in
order [1,2,3,0]: block J+1's LN1/Q/V issue before block J's attention, and the
RS-dependent residual work is deferred two stages so no in-order engine queue
ever stalls on the collective. Transposes run on the DMA crossbar
(dma_start_transpose), the key-mask bias is folded into the score matmul via an
augmented contraction row (so exp needs no per-chunk bias and two score chunks
share one activation), diagonal score chunks only compute the causally valid
columns, LN uses Ln/Exp for the rsqrt (keeps the scalar activation table on
one function set), and the softmax divide uses reciprocal_approx_fast plus a
fused PSUM multiply.
"""

import sys

if "/opt/trn_rl_repo" not in sys.path:
    sys.path.insert(0, "/opt/trn_rl_repo")

import numpy as np

import concourse.bacc as bacc
import concourse.bass_utils as _bass_utils
import concourse.mybir as mybir
import concourse.tile as tile
from concourse.bass_utils import run_bass_kernel_spmd

import os as _os

if _os.environ.get("KERNEL_LDW_OPT", "0") == "1":
    # Overlap LDWEIGHTS with in-flight matmuls; saves the serialized
    # weight-load slot per matmul. All transposes in this kernel run on the
    # DMA crossbar, so no transpose-ldweights blocks the walrus LDW pass.
    _orig_run_command = _bass_utils.run_command

    def _run_command_ldwopt(cmd, *a, **kw):
        cmd = ["--enable-ldw-opt=true" if c == "--enable-ldw-opt=false" else c
               for c in cmd]
        return _orig_run_command(cmd, *a, **kw)

    _bass_utils.run_command = _run_command_ldwopt

B, S, E, H, D, F = 2, 2048, 768, 12, 64, 3072
NCORES = 8
R = 4          # cores per batch group
HPC = 3        # heads per core
MYR = S // R   # rows per core after reduce-scatter (512)
EC = E // 128  # 6 e-chunks
SC = S // 128  # 16 s-chunks of 128
FC = F // 128  # 24 f-chunks
VW = 256       # padded V width (3 heads x 65 = 195 -> 256)

f32 = mybir.dt.float32
f16 = mybir.dt.float16
AF = mybir.ActivationFunctionType
ALU = mybir.AluOpType

MASK_BIAS = -50.0
EXP_SHIFT = -8.0  # uniform exp shift; cancels in softmax, keeps fp16 in range
SCALE = 0.125     # 1/sqrt(D)

# mega-packed constant layouts (columns)
C16 = {"wq": (0, EC * 192), "wv": (1152, EC * VW), "wo0": (2688, E),
       "wo1": (3456, E), "m_band": (4224, 128), "dd_band": (4352, 16 * 128)}
C16N = 6400
C32 = {"bqc": (0, 2), "bv_bc": (2, VW), "bo_bc": (258, E), "b1c": (1026, FC),
       "b2c": (1050, FC), "b3_bc": (1074, E)}
C32N = 1842

_CACHE = {}


def _declare_io(nc):
    t = {}

    F16_INPUTS = {"cf16", "mbrow", "ones_s", "ones_row", "w1", "w2", "w3"}

    def inp(name, shape):
        dt = f16 if name in F16_INPUTS else f32
        t[name] = nc.dram_tensor(name, list(shape), dt, kind="ExternalInput").ap()

    inp("xb", (128, SC * E))          # x[b] rows-chunked
    inp("xmy", (128, 4 * E))          # my 512 rows of x[b]
    inp("cf16", (128, C16N))          # packed f16 consts
    inp("cf32", (128, C32N))          # packed f32 consts
    inp("mbrow", (1, S))              # key mask bias / SCALE, one row
    inp("ones_s", (1, S))             # ones row (query-side matmul augmentation)
    inp("ones_row", (1, 512))         # ones (recip broadcast matmul lhsT)
    inp("w1", (128, EC * F))          # W1_eff e-chunked
    inp("w2", (128, FC * F))          # W2 f1-chunked
    inp("w3", (128, FC * E))          # W3 f-chunked
    t["out"] = nc.dram_tensor("out", [128, 4 * E], f32, kind="ExternalOutput").ap()
    return t


def _layernorm_chunk(nc, pool, x_chunk, out_chunk):
    """LN a [128, 768] fp32 chunk into out_chunk (f16), eps=1e-5.

    The normalize itself runs on the vector engine."""
    stats = pool.tile([128, 12], f32, tag="ln_stats")
    nc.vector.bn_stats(stats[:, 0:6], x_chunk[:, 0:384])
    nc.vector.bn_stats(stats[:, 6:12], x_chunk[:, 384:768])
    mv = pool.tile([128, 2], f32, tag="ln_mv")
    nc.vector.bn_aggr(mv[:], stats[:])
    veps = pool.tile([128, 1], f32, tag="ln_veps")
    nc.vector.tensor_scalar_add(veps[:], mv[:, 1:2], 1e-5)
    sig = pool.tile([128, 1], f32, tag="ln_sig")
    nc.scalar.sqrt(sig[:], veps[:])
    rsig = pool.tile([128, 1], f32, tag="ln_rsig")
    nc.vector.reciprocal(rsig[:], sig[:])
    negmu = pool.tile([128, 1], f32, tag="ln_negmu")
    nc.vector.tensor_scalar_mul(negmu[:], mv[:, 0:1], -1.0)
    nc.vector.tensor_scalar(
        out_chunk, x_chunk, negmu[:], rsig[:], ALU.add, ALU.mult
    )


def _build_body(tc, t):
    nc = tc.nc

    with tc.tile_pool(name="constp", bufs=1) as constp, \
         tc.tile_pool(name="lnstat", bufs=4) as lnstat, \
         tc.tile_pool(name="dramp", bufs=1, space="DRAM") as dramp, \
         tc.tile_pool(name="w1p", bufs=3) as w1p, \
         tc.tile_pool(name="w2p", bufs=3) as w2p, \
         tc.tile_pool(name="w3p", bufs=4) as w3p, \
         tc.tile_pool(name="yp", bufs=1) as yp:
        # per-row-block bounce buffers for the chunked reduce-scatter (f16)
        proj_J = [dramp.tile([MYR, E], f16, name=f"projb_{J}") for J in range(4)]
        rs_J = [dramp.tile([128, E], f16, name=f"rsout_{J}") for J in range(4)]

        # x blocks first on the DMA queue, then the two packed const loads
        xp_tiles = {}
        xb3 = t["xb"][:].rearrange("p (s e) -> p s e", s=SC)

        cf16 = constp.tile([128, C16N], f16)
        cf32 = constp.tile([128, C32N], f32)
        ones_row = constp.tile([1, 512], f16)
        eshift = constp.tile([128, 1], f32)

        def cslice(tile_, table, name):
            o, n = table[name]
            return tile_[:, o:o + n]

        wq3 = cslice(cf16, C16, "wq").rearrange("p (e m) -> p e m", e=EC)
        wv3 = cslice(cf16, C16, "wv").rearrange("p (e m) -> p e m", e=EC)
        wo0 = cslice(cf16, C16, "wo0")
        wo1 = cslice(cf16, C16, "wo1")[0:64, :]
        m_band = cslice(cf16, C16, "m_band")
        dd3 = cslice(cf16, C16, "dd_band").rearrange("p (j c) -> p j c", j=16)
        bqc = cslice(cf32, C32, "bqc")
        bv_bc = cslice(cf32, C32, "bv_bc")
        bo_bc = cslice(cf32, C32, "bo_bc")
        b1c = cslice(cf32, C32, "b1c")
        b2c = cslice(cf32, C32, "b2c")
        b3_bc = cslice(cf32, C32, "b3_bc")

        # long-lived FFN-input tiles
        y1 = yp.tile([128, 4 * E], f32)
        y13 = y1[:].rearrange("p (c e) -> p c e", c=4)
        ylnT = yp.tile([128, EC * MYR], f16)
        ylnT3 = ylnT[:].rearrange("p (e s) -> p e s", e=EC)
        xmy_sb = yp.tile([128, 4 * E], f32)
        xmy3 = xmy_sb[:].rearrange("p (c e) -> p c e", c=4)

        with tc.tile_pool(name="attnp", bufs=1) as attnp, \
             tc.tile_pool(name="xp", bufs=2) as xp, \
             tc.tile_pool(name="ln1Tp", bufs=2) as ln1Tp, \
             tc.tile_pool(name="lnxp", bufs=2) as lnxp, \
             tc.tile_pool(name="pexp", bufs=3) as pexp, \
             tc.tile_pool(name="projp", bufs=1) as projp, \
             tc.tile_pool(name="zp", bufs=2) as zp, \
             tc.tile_pool(name="lnyp", bufs=2) as lnyp, \
             tc.tile_pool(name="psP", bufs=2, space="PSUM") as psP, \
             tc.tile_pool(name="psO", bufs=2, space="PSUM") as psO, \
             tc.tile_pool(name="psM", bufs=2, space="PSUM") as psM:

            def fetch_x(J):
                x_blk = xp.tile([128, 4 * E], f32, tag="xblk",
                                name=f"xblk_{J}")
                xp_tiles[J] = x_blk[:].rearrange("p (s e) -> p s e", s=4)
                nc.sync.dma_start(
                    xp_tiles[J][:, :, :], xb3[:, 4 * J:4 * (J + 1), :])

            fetch_x(0)
            nc.sync.dma_start(cf16[:], t["cf16"][:])
            nc.sync.dma_start(cf32[:], t["cf32"][:])
            fetch_x(1)
            nc.sync.dma_start(ones_row[:], t["ones_row"][:])
            nc.vector.memset(eshift[:], EXP_SHIFT)

            # Per-head q tensors with a 65th contraction row:
            # qa (key side): row 64 = key-mask bias / SCALE
            # qp (query side): row 64 = 1.0
            qa = [attnp.tile([65, S], f16, name=f"qa_{h}") for h in range(HPC)]
            qp = [attnp.tile([65, S], f16, name=f"qp_{h}") for h in range(HPC)]
            v_sb = attnp.tile([128, SC * VW], f16)
            v3 = v_sb[:].rearrange("p (s v) -> p s v", s=SC)
            oT01 = attnp.tile([128, S], f16)
            oT2 = attnp.tile([64, S], f16)

            ln_views = {}

            def ln1_qv(J, prefetch):
                # ---- LN1 + one DMA-crossbar transpose for token block J ----
                x3 = xp_tiles[J]
                lnx = lnxp.tile([128, 4 * E], f16, tag="lnx")
                for k in range(4):
                    _layernorm_chunk(nc, lnstat, x3[:, k, :],
                                     lnx[:, k * E:(k + 1) * E])
                ln1T_J = ln1Tp.tile([128, 4 * EC * 128], f16, tag="ln1T")
                nc.sync.dma_start_transpose(
                    ln1T_J[:].rearrange("p (b s) -> p b s", b=4 * EC), lnx[:])
                if prefetch is not None:
                    fetch_x(prefetch)
                # layout [p, k, e, s]: token kk*128+ss lives at [:, kk, :, ss]
                ln4 = ln1T_J[:].rearrange("p (k e s) -> p k e s", k=4, e=EC)
                ln_views[J] = ln4

                # ---- Q for block J (writes both qa and qp data rows) ----
                for g in range(2):
                    m = 128 if g == 0 else 64
                    pq = psM.tile([128, 512], f32, tag="pm")
                    for ec in range(EC):
                        nc.tensor.matmul(
                            pq[:m, :],
                            wq3[:, ec, g * 128:g * 128 + m],
                            ln4[:, :, ec, :],
                            start=(ec == 0), stop=(ec == EC - 1),
                        )
                    Jc = slice(J * 512, (J + 1) * 512)
                    if g == 0:
                        nc.vector.tensor_scalar_add(
                            qa[0][0:64, Jc], pq[0:64, :], bqc[0:64, 0:1])
                        nc.vector.tensor_scalar_add(
                            qp[0][0:64, Jc], pq[0:64, :], bqc[0:64, 0:1])
                        nc.vector.tensor_scalar_add(
                            qa[1][0:64, Jc], pq[64:128, :], bqc[64:128, 0:1])
                        nc.vector.tensor_scalar_add(
                            qp[1][0:64, Jc], pq[64:128, :], bqc[64:128, 0:1])
                    else:
                        nc.vector.tensor_scalar_add(
                            qa[2][0:64, Jc], pq[0:64, :], bqc[0:64, 1:2])
                        nc.vector.tensor_scalar_add(
                            qp[2][0:64, Jc], pq[0:64, :], bqc[0:64, 1:2])

                # ---- V for the 4 s-chunks of block J ----
                for k in range(4):
                    pv = psM.tile([128, VW], f32, tag="pm")
                    for ec in range(EC):
                        nc.tensor.matmul(
                            pv[:, :],
                            ln4[:, k, ec, :],
                            wv3[:, ec, :],
                            start=(ec == 0), stop=(ec == EC - 1),
                        )
                    nc.vector.tensor_tensor(
                        v3[:, 4 * J + k, :], pv[:, :], bv_bc[:], ALU.add
                    )

            def attn_wo_rs(J):
                # ---- attention for block J ----
                ntc = 4 * J + 4
                for hh in range(HPC):
                    oT = oT01 if hh < 2 else oT2
                    op_off = 64 * hh if hh < 2 else 0
                    po = psO.tile([128, 512], f32, tag="po")
                    for pr in range(ntc // 2):
                        tc0, tc1 = 2 * pr, 2 * pr + 1
                        k0, k1 = tc0 - 4 * J, tc1 - 4 * J
                        off0 = 128 * k0 if k0 > 0 else 0
                        off1 = 128 * k1 if k1 > 0 else 0
                        w0, w1_ = 512 - off0, 512 - off1
                        pp = psP.tile([128, 1024], f32, tag="pp")
                        nc.tensor.matmul(
                            pp[:, 0:w0],
                            qa[hh][:, tc0 * 128:(tc0 + 1) * 128],
                            qp[hh][:, J * 512 + off0:(J + 1) * 512],
                            start=True, stop=True,
                        )
                        nc.tensor.matmul(
                            pp[:, w0:w0 + w1_],
                            qa[hh][:, tc1 * 128:(tc1 + 1) * 128],
                            qp[hh][:, J * 512 + off1:(J + 1) * 512],
                            start=True, stop=True,
                        )
                        pe = pexp.tile([128, 1024], f16, tag="pe")
                        nc.scalar.activation(
                            pe[:, 0:w0 + w1_], pp[:, 0:w0 + w1_], AF.Exp,
                            bias=eshift[:], scale=SCALE,
                        )
                        if k0 >= 0:
                            nc.vector.tensor_mul(
                                pe[:, 0:128], pe[:, 0:128], m_band[:])
                            nc.vector.tensor_add(
                                pe[:, 0:128], pe[:, 0:128], dd3[:, 4 * J + k0, :])
                        if k1 >= 0:
                            nc.vector.tensor_mul(
                                pe[:, w0:w0 + 128], pe[:, w0:w0 + 128], m_band[:])
                            nc.vector.tensor_add(
                                pe[:, w0:w0 + 128], pe[:, w0:w0 + 128],
                                dd3[:, 4 * J + k1, :])
                        hs = slice(65 * hh, 65 * hh + 65)
                        nc.tensor.matmul(
                            po[:65, off0:512], v3[:, tc0, hs], pe[:, 0:w0],
                            start=(tc0 == 0), stop=False,
                        )
                        nc.tensor.matmul(
                            po[:65, off1:512], v3[:, tc1, hs], pe[:, w0:w0 + w1_],
                            start=False, stop=(tc1 == ntc - 1),
                        )
                    # softmax divide: oT[:, Jblock] = po[:64] * (1/po[64])
                    # (copy Z to partition 0 first: reciprocal_approx_fast
                    # mis-addresses inputs with a nonzero base partition)
                    zcp = zp.tile([1, 512], f32, tag="zcp")
                    nc.vector.tensor_copy(zcp[:], po[64:65, :])
                    zr = zp.tile([1, 512], f32, tag="zr")
                    nc.vector.reciprocal_approx_fast(zr[:], zcp[:])
                    zrech = zp.tile([1, 512], f16, tag="zrech")
                    nc.vector.tensor_copy(zrech[:], zr[:])
                    pb = psM.tile([64, 512], f32, tag="pm")
                    nc.tensor.matmul(
                        pb[:], ones_row[0:1, 0:64], zrech[:],
                        start=True, stop=True,
                    )
                    pb_sb = zp.tile([64, 512], f16, tag="pb_sb")
                    nc.scalar.copy(pb_sb[:], pb[:])
                    nc.vector.tensor_tensor(
                        oT[op_off:op_off + 64, J * 512:(J + 1) * 512],
                        po[0:64, :], pb_sb[:], ALU.mult,
                    )

                # ---- Wo projection for this row block + partial RS ----
                prj = projp.tile([128, 4 * E], f16, tag="prj")
                prj3 = prj[:].rearrange("p (s e) -> p s e", s=4)
                for sl in range(4):
                    st = 4 * J + sl
                    for hf in range(2):
                        pw = psM.tile([128, 384], f32, tag="pm")
                        nc.tensor.matmul(
                            pw[:],
                            oT01[:, st * 128:(st + 1) * 128],
                            wo0[:, hf * 384:(hf + 1) * 384],
                            start=True, stop=False,
                        )
                        nc.tensor.matmul(
                            pw[:],
                            oT2[:, st * 128:(st + 1) * 128],
                            wo1[:, hf * 384:(hf + 1) * 384],
                            start=False, stop=True,
                        )
                        nc.vector.tensor_tensor(
                            prj3[:, sl, hf * 384:(hf + 1) * 384], pw[:],
                            bo_bc[:, hf * 384:(hf + 1) * 384], ALU.add
                        )
                nc.sync.dma_start(
                    proj_J[J][:, :].rearrange("(s p) e -> p s e", s=4), prj3)
                nc.gpsimd.collective_compute(
                    "ReduceScatter",
                    ALU.add,
                    replica_groups=[[0, 1, 2, 3], [4, 5, 6, 7]],
                    ins=[proj_J[J][:, :].opt()],
                    outs=[rs_J[J][:, :].opt()],
                )

            def finish(J):
                # residual + LN2 for the 128 rows this core received
                rs_sb = yp.tile([128, E], f16, tag="rs_sb",
                                name=f"rs_sb_{J}", bufs=4)
                nc.sync.dma_start(rs_sb[:], rs_J[J][:, :])
                nc.vector.tensor_add(y13[:, J, :], rs_sb[:], xmy3[:, J, :])
                lny = lnyp.tile([128, E], f16, tag="lny")
                _layernorm_chunk(nc, lnstat, y13[:, J, :], lny[:])
                # b3 bypasses LN2: add into y1 after LN2 consumed it
                nc.vector.tensor_tensor(
                    y13[:, J, :], y13[:, J, :], b3_bc[:], ALU.add
                )
                nc.sync.dma_start_transpose(
                    ylnT3[:, :, J * 128:(J + 1) * 128], lny[:]
                )

            # software pipeline, blocks processed [1,2,3,0]: the last block's
            # attention is the cheapest so its RS tail before the FFN is
            # short; each finish runs two stages after its RS was issued.
            ln1_qv(0, prefetch=2)
            for h in range(HPC):
                nc.sync.dma_start(qa[h][64:65, :], t["mbrow"][:])
                nc.sync.dma_start(qp[h][64:65, :], t["ones_s"][:])
            ln1_qv(1, prefetch=3)
            nc.sync.dma_start(xmy_sb[:], t["xmy"][:])
            ln1_qv(2, prefetch=None)
            attn_wo_rs(1)
            ln1_qv(3, prefetch=None)
            attn_wo_rs(2)
            finish(1)
            attn_wo_rs(3)
            finish(2)
            attn_wo_rs(0)
            finish(3)
            finish(0)

        # ---------------- Phase E: FFN on my 512 rows ----------------------
        w13 = t["w1"][:].rearrange("p (e f) -> p e f", e=EC)
        w23 = t["w2"][:].rearrange("p (f g) -> p f g", f=FC)
        w33 = t["w3"][:].rearrange("p (f e) -> p f e", f=FC)

        with tc.tile_pool(name="ffp", bufs=1) as ffp:
            h1T = ffp.tile([128, FC * MYR], f16)
            h13 = h1T[:].rearrange("p (f s) -> p f s", f=FC)
            h2T = ffp.tile([128, FC * MYR], f16)
            h23 = h2T[:].rearrange("p (f s) -> p f s", f=FC)

            with tc.tile_pool(name="psF1", bufs=2, space="PSUM") as psF1, \
                 tc.tile_pool(name="psF2", bufs=4, space="PSUM") as psF2:
                # h1T = relu(W1^T @ yln^T + b1), in two phases: token
                # blocks 1-3 first (their ylnT cols land before the last
                # reduce-scatter), then block 0 (w1 tiles re-streamed)
                for phase, lo, hi in ((0, 128, 512), (1, 0, 128)):
                    for fc2 in range(FC // 2):
                        w1t = w1p.tile([128, EC * 256], f16, tag="w1t")
                        w1t3 = w1t[:].rearrange("p (e f) -> p e f", e=EC)
                        nc.sync.dma_start(
                            w1t3, w13[:, :, fc2 * 256:(fc2 + 1) * 256])
                        for j in range(2):
                            fc = 2 * fc2 + j
                            pf = psF1.tile([128, hi - lo], f32,
                                           tag=f"pf{phase}")
                            for ec in range(EC):
                                nc.tensor.matmul(
                                    pf[:],
                                    w1t3[:, ec, j * 128:(j + 1) * 128],
                                    ylnT3[:, ec, lo:hi],
                                    start=(ec == 0), stop=(ec == EC - 1),
                                )
                            nc.scalar.activation(
                                h13[:, fc, lo:hi], pf[:], AF.Relu,
                                bias=b1c[:, fc:fc + 1], scale=1.0,
                            )
                # h2T = relu(W2^T @ h1T + b2)
                for f2b in range(6):
                    ph2 = [
                        psF2.tile([128, 512], f32, tag="ph2", name=f"ph2_{f2b}_{k}")
                        for k in range(4)
                    ]
                    for f1c4 in range(FC // 4):
                        w2t = w2p.tile([128, 4 * 512], f16, tag="w2t")
                        w2t3 = w2t[:].rearrange("p (j g) -> p j g", j=4)
                        nc.sync.dma_start(
                            w2t3,
                            w23[:, 4 * f1c4:4 * f1c4 + 4,
                                f2b * 512:(f2b + 1) * 512],
                        )
                        for j in range(4):
                            f1c = 4 * f1c4 + j
                            for k in range(4):
                                nc.tensor.matmul(
                                    ph2[k][:],
                                    w2t3[:, j, k * 128:(k + 1) * 128],
                                    h13[:, f1c, :],
                                    start=(f1c == 0), stop=(f1c == FC - 1),
                                )
                    for k in range(4):
                        fc2 = f2b * 4 + k
                        nc.scalar.activation(
                            h23[:, fc2, :], ph2[k][:], AF.Relu,
                            bias=b2c[:, fc2:fc2 + 1], scale=1.0,
                        )

            # out = h2 @ W3 + b3 + y1
            with tc.tile_pool(name="outp", bufs=1) as outp, \
                 tc.tile_pool(name="psF3", bufs=1, space="PSUM") as psF3:
                out_sb = outp.tile([128, 4 * E], f32)
                out3 = out_sb[:].rearrange("p (c e) -> p c e", c=4)
                p3 = [
                    psF3.tile([128, 384], f32, tag=f"p3_{st}_{hf}",
                              name=f"p3_{st}_{hf}")
                    for st in range(4) for hf in range(2)
                ]
                for fc2 in range(FC // 2):
                    w3t = w3p.tile([128, 2 * E], f16, tag="w3t")
                    w3t3 = w3t[:].rearrange("p (j e) -> p j e", j=2)
                    nc.sync.dma_start(
                        w3t3, w33[:, 2 * fc2:2 * fc2 + 2, :])
                    for j in range(2):
                        fc = 2 * fc2 + j
                        for st in range(4):
                            for hf in range(2):
                                nc.tensor.matmul(
                                    p3[st * 2 + hf][:],
                                    h23[:, fc, st * 128:(st + 1) * 128],
                                    w3t3[:, j, hf * 384:(hf + 1) * 384],
                                    start=(fc == 0), stop=(fc == FC - 1),
                                )
                for st in range(4):
                    for hf in range(2):
                        nc.vector.tensor_add(
                            out3[:, st, hf * 384:(hf + 1) * 384],
                            p3[st * 2 + hf][:],
                            y13[:, st, hf * 384:(hf + 1) * 384],
                        )
                nc.sync.dma_start(t["out"][:], out_sb[:])


def _build():
    if "nc" in _CACHE:
        return _CACHE["nc"]
    nc = bacc.Bacc("TRN2", target_bir_lowering=False, debug=False,
                   num_devices=NCORES)
    t = _declare_io(nc)
    with tile.TileContext(nc) as tc:
        _build_body(tc, t)
    nc.compile()
    _CACHE["nc"] = nc
    return nc


def _chunk_rows(a, p=128):
    """[N, M] -> [p, N//p, M] -> [p, (N//p)*M] row-chunk packing."""
    n, m = a.shape
    return np.ascontiguousarray(
        a.reshape(n // p, p, m).transpose(1, 0, 2).reshape(p, -1)
    )


def _prep_in_maps(inputs):
    x = np.asarray(inputs["x"], np.float32)
    Wq = np.asarray(inputs["Wq"], np.float32)
    bq = np.asarray(inputs["bq"], np.float32)
    Wv = np.asarray(inputs["Wv"], np.float32)
    bv = np.asarray(inputs["bv"], np.float32)
    Wo = np.asarray(inputs["Wo"], np.float32)
    bo = np.asarray(inputs["bo"], np.float32)
    ln1_g = np.asarray(inputs["ln1_g"], np.float32)
    ln1_b = np.asarray(inputs["ln1_b"], np.float32)
    W1 = np.asarray(inputs["W1"], np.float32)
    b1 = np.asarray(inputs["b1"], np.float32)
    W2 = np.asarray(inputs["W2"], np.float32)
    b2 = np.asarray(inputs["b2"], np.float32)
    W3 = np.asarray(inputs["W3"], np.float32)
    b3 = np.asarray(inputs["b3"], np.float32)
    ln2_g = np.asarray(inputs["ln2_g"], np.float32)
    ln2_b = np.asarray(inputs["ln2_b"], np.float32)
    mask = np.asarray(inputs["input_mask"])

    # Fold LN affine params into the following projections (exact algebra).
    Wq_eff = Wq * ln1_g[None, :, None]
    bq_eff = bq + np.einsum("e,hed->hd", ln1_b, Wq)
    Wv_eff = Wv * ln1_g[None, :, None]
    bv_eff = bv + np.einsum("e,hed->hd", ln1_b, Wv)
    W1_eff = W1 * ln2_g[:, None]
    b1_eff = b1 + ln2_b @ W1

    w1_p = _chunk_rows(W1_eff)                      # [128, 6*3072]
    b1c = np.ascontiguousarray(b1_eff.reshape(FC, 128).T)
    w2_p = _chunk_rows(W2)                          # [128, 24*3072]
    b2c = np.ascontiguousarray(b2.reshape(FC, 128).T)
    w3_p = _chunk_rows(W3)                          # [128, 24*768]
    b3_bc = np.broadcast_to(b3.astype(np.float32), (128, E)).copy()
    ones_row = np.ones((1, 512), np.float32)
    ones_s = np.ones((1, S), np.float32)

    # causal 0/1 band mask for the diagonal 128x128 band of each diag chunk
    tl = np.arange(128)[:, None]
    cl = np.arange(128)[None, :]
    m_band = (cl >= tl).astype(np.float16)

    f16c = np.float16
    in_maps = []
    for c in range(NCORES):
        b, r = c // R, c % R
        hs = [HPC * r + i for i in range(HPC)]

        xb = _chunk_rows(x[b])                      # [128, 16*768]
        myrows = np.concatenate(
            [np.arange(512 * J + 128 * r, 512 * J + 128 * r + 128)
             for J in range(4)]
        )
        xmy = _chunk_rows(x[b, myrows])

        Wq_my = np.concatenate([Wq_eff[h] for h in hs], axis=1)   # [E, 192]
        bq_my = np.concatenate([bq_eff[h] for h in hs])           # [192]
        wq_p = _chunk_rows(Wq_my)
        bqc = np.zeros((128, 2), np.float32)
        bqc[:, 0] = bq_my[:128]
        bqc[:64, 1] = bq_my[128:]

        Wv_aug = np.zeros((E, VW), np.float32)
        bv1 = np.zeros((1, VW), np.float32)
        for i, h in enumerate(hs):
            Wv_aug[:, 65 * i: 65 * i + 64] = Wv_eff[h]
            bv1[0, 65 * i: 65 * i + 64] = bv_eff[h]
            bv1[0, 65 * i + 64] = 1.0
        bv_bc = np.broadcast_to(bv1, (128, VW)).copy()
        wv_p = _chunk_rows(Wv_aug)

        wo0 = np.ascontiguousarray(Wo[hs[0] * D: hs[0] * D + 128])
        wo1 = np.zeros((128, E), np.float32)
        wo1[0:64] = Wo[hs[2] * D: hs[2] * D + 64]
        bo_bc = np.broadcast_to(
            (bo if r == 0 else np.zeros_like(bo)).astype(np.float32), (128, E)
        ).copy()

        # key-mask bias row, pre-divided by the exp scale (applied via the
        # augmented contraction row in the score matmul)
        mbrow = (np.where(mask[b] == 0, MASK_BIAS / SCALE, 0.0)
                 .astype(np.float32)[None, :])

        bad = (np.cumsum(mask[b]) == 0).astype(np.float32) * \
            np.float32(np.exp(EXP_SHIFT))
        dd_b = np.zeros((128, 16, 128), np.float16)
        i128 = np.arange(128)
        for jk in range(16):
            dd_b[i128, jk, i128] = bad[jk * 128 + i128]

        cf16 = np.zeros((128, C16N), np.float16)
        for name, arr in [("wq", wq_p), ("wv", wv_p), ("wo0", wo0),
                          ("wo1", wo1), ("m_band", m_band),
                          ("dd_band", dd_b.reshape(128, -1))]:
            o, n = C16[name]
            cf16[:, o:o + n] = arr.astype(f16c)
        cf32 = np.zeros((128, C32N), np.float32)
        for name, arr in [("bqc", bqc), ("bv_bc", bv_bc), ("bo_bc", bo_bc),
                          ("b1c", b1c), ("b2c", b2c), ("b3_bc", b3_bc)]:
            o, n = C32[name]
            cf32[:, o:o + n] = arr

        in_maps.append({
            "xb": xb, "xmy": xmy,
            "cf16": cf16, "cf32": cf32,
            "mbrow": mbrow.astype(f16c),
            "ones_s": ones_s.astype(f16c),
            "ones_row": ones_row.astype(f16c),
            "w1": w1_p.astype(f16c),
            "w2": w2_p.astype(f16c),
            "w3": w3_p.astype(f16c),
        })
    return in_maps


def _gather(results):
    y = np.empty((B, S, E), np.float32)
    for c in range(NCORES):
        b, r = c // R, c % R
        o = results[c]["out"].reshape(128, 4, E).transpose(1, 0, 2).reshape(MYR, E)
        myrows = np.concatenate(
            [np.arange(512 * J + 128 * r, 512 * J + 128 * r + 128)
             for J in range(4)]
        )
        y[b, myrows] = o
    return y


def run(inputs, **spmd_kwargs):
    nc = _build()
    in_maps = _prep_in_maps(inputs)
    res = run_bass_kernel_spmd(nc, in_maps, core_ids=list(range(NCORES)),
                               **spmd_kwargs)
    return _gather(res.results), res


def kernel(**inputs) -> np.ndarray:
    y, _ = run(inputs)
    return y


# revision 28
# speedup vs baseline: 1.0217x; 1.0217x over previous
"""Trainium2 Bass kernel for a dense transformer block (B=2,S=2048,E=768,H=12,D=64,F=3072).

Sharding: 8 cores = 2 batch groups x 4 cores. Within a batch group each core
computes attention for 3 of the 12 heads over the full sequence, partial output
projections are combined with a 4-core ReduceScatter (f16 payload), and each
core then runs the FFN on its 512 rows with replicated FFN weights.

The pre-FFN section is software-pipelined over 512-token blocks processed in
order [1,2,3,0]: block J+1's LN1/Q/V issue before block J's attention, and the
RS-dependent residual work is deferred two stages so no in-order engine queue
ever stalls on the collective. Transposes run on the DMA crossbar
(dma_start_transpose), the key-mask bias is folded into the score matmul via an
augmented contraction row (so exp needs no per-chunk bias and two score chunks
share one activation), diagonal score chunks only compute the causally valid
columns, LN uses Ln/Exp for the rsqrt (keeps the scalar activation table on
one function set), and the softmax divide uses reciprocal_approx_fast plus a
fused PSUM multiply.
"""

import sys

if "/opt/trn_rl_repo" not in sys.path:
    sys.path.insert(0, "/opt/trn_rl_repo")

import numpy as np

import concourse.bacc as bacc
import concourse.bass_utils as _bass_utils
import concourse.mybir as mybir
import concourse.tile as tile
from concourse.bass_utils import run_bass_kernel_spmd

import os as _os

if _os.environ.get("KERNEL_LDW_OPT", "0") == "1":
    # Overlap LDWEIGHTS with in-flight matmuls; saves the serialized
    # weight-load slot per matmul. All transposes in this kernel run on the
    # DMA crossbar, so no transpose-ldweights blocks the walrus LDW pass.
    _orig_run_command = _bass_utils.run_command

    def _run_command_ldwopt(cmd, *a, **kw):
        cmd = ["--enable-ldw-opt=true" if c == "--enable-ldw-opt=false" else c
               for c in cmd]
        return _orig_run_command(cmd, *a, **kw)

    _bass_utils.run_command = _run_command_ldwopt

B, S, E, H, D, F = 2, 2048, 768, 12, 64, 3072
NCORES = 8
R = 4          # cores per batch group
HPC = 3        # heads per core
MYR = S // R   # rows per core after reduce-scatter (512)
EC = E // 128  # 6 e-chunks
SC = S // 128  # 16 s-chunks of 128
FC = F // 128  # 24 f-chunks
VW = 256       # padded V width (3 heads x 65 = 195 -> 256)

f32 = mybir.dt.float32
f16 = mybir.dt.float16
AF = mybir.ActivationFunctionType
ALU = mybir.AluOpType

MASK_BIAS = -50.0
EXP_SHIFT = -8.0  # uniform exp shift; cancels in softmax, keeps fp16 in range
SCALE = 0.125     # 1/sqrt(D)

# mega-packed constant layouts (columns)
C16 = {"wq": (0, EC * 192), "wv": (1152, EC * VW), "wo0": (2688, E),
       "wo1": (3456, E), "m_band": (4224, 128), "dd_band": (4352, 16 * 128)}
C16N = 6400
C32 = {"bqc": (0, 2), "bv_bc": (2, VW), "bo_bc": (258, E), "b1c": (1026, FC),
       "b2c": (1050, FC), "b3_bc": (1074, E)}
C32N = 1842

_CACHE = {}


def _declare_io(nc):
    t = {}

    F16_INPUTS = {"cf16", "mbrow", "ones_s", "ones_row", "w1", "w2", "w3"}

    def inp(name, shape):
        dt = f16 if name in F16_INPUTS else f32
        t[name] = nc.dram_tensor(name, list(shape), dt, kind="ExternalInput").ap()

    inp("xb", (128, SC * E))          # x[b] rows-chunked
    inp("xmy", (128, 4 * E))          # my 512 rows of x[b]
    inp("cf16", (128, C16N))          # packed f16 consts
    inp("cf32", (128, C32N))          # packed f32 consts
    inp("mbrow", (1, S))              # key mask bias / SCALE, one row
    inp("ones_s", (1, S))             # ones row (query-side matmul augmentation)
    inp("ones_row", (1, 512))         # ones (recip broadcast matmul lhsT)
    inp("w1", (128, EC * F))          # W1_eff e-chunked
    inp("w2", (128, FC * F))          # W2 f1-chunked
    inp("w3", (128, FC * E))          # W3 f-chunked
    t["out"] = nc.dram_tensor("out", [128, 4 * E], f32, kind="ExternalOutput").ap()
    return t


def _layernorm_chunk(nc, pool, x_chunk, out_chunk):
    """LN a [128, 768] fp32 chunk into out_chunk (f16), eps=1e-5.

    The normalize itself runs on the vector engine."""
    stats = pool.tile([128, 12], f32, tag="ln_stats")
    nc.vector.bn_stats(stats[:, 0:6], x_chunk[:, 0:384])
    nc.vector.bn_stats(stats[:, 6:12], x_chunk[:, 384:768])
    mv = pool.tile([128, 2], f32, tag="ln_mv")
    nc.vector.bn_aggr(mv[:], stats[:])
    veps = pool.tile([128, 1], f32, tag="ln_veps")
    nc.vector.tensor_scalar_add(veps[:], mv[:, 1:2], 1e-5)
    sig = pool.tile([128, 1], f32, tag="ln_sig")
    nc.scalar.sqrt(sig[:], veps[:])
    rsig = pool.tile([128, 1], f32, tag="ln_rsig")
    nc.vector.reciprocal(rsig[:], sig[:])
    negmu = pool.tile([128, 1], f32, tag="ln_negmu")
    nc.vector.tensor_scalar_mul(negmu[:], mv[:, 0:1], -1.0)
    nc.vector.tensor_scalar(
        out_chunk, x_chunk, negmu[:], rsig[:], ALU.add, ALU.mult
    )


def _build_body(tc, t):
    nc = tc.nc

    with tc.tile_pool(name="constp", bufs=1) as constp, \
         tc.tile_pool(name="lnstat", bufs=4) as lnstat, \
         tc.tile_pool(name="dramp", bufs=1, space="DRAM") as dramp, \
         tc.tile_pool(name="w1p", bufs=3) as w1p, \
         tc.tile_pool(name="w2p", bufs=3) as w2p, \
         tc.tile_pool(name="yp", bufs=1) as yp, \
         tc.tile_pool(name="lnyp", bufs=2) as lnyp:
        # per-row-block bounce buffers for the chunked reduce-scatter (f16)
        proj_J = [dramp.tile([MYR, E], f16, name=f"projb_{J}") for J in range(4)]
        rs_J = [dramp.tile([128, E], f16, name=f"rsout_{J}") for J in range(4)]

        # x blocks first on the DMA queue, then the two packed const loads
        xp_tiles = {}
        xb3 = t["xb"][:].rearrange("p (s e) -> p s e", s=SC)

        cf16 = constp.tile([128, C16N], f16)
        cf32 = constp.tile([128, C32N], f32)
        ones_row = constp.tile([1, 512], f16)
        eshift = constp.tile([128, 1], f32)

        def cslice(tile_, table, name):
            o, n = table[name]
            return tile_[:, o:o + n]

        wq3 = cslice(cf16, C16, "wq").rearrange("p (e m) -> p e m", e=EC)
        wv3 = cslice(cf16, C16, "wv").rearrange("p (e m) -> p e m", e=EC)
        wo0 = cslice(cf16, C16, "wo0")
        wo1 = cslice(cf16, C16, "wo1")[0:64, :]
        m_band = cslice(cf16, C16, "m_band")
        dd3 = cslice(cf16, C16, "dd_band").rearrange("p (j c) -> p j c", j=16)
        bqc = cslice(cf32, C32, "bqc")
        bv_bc = cslice(cf32, C32, "bv_bc")
        bo_bc = cslice(cf32, C32, "bo_bc")
        b1c = cslice(cf32, C32, "b1c")
        b2c = cslice(cf32, C32, "b2c")
        b3_bc = cslice(cf32, C32, "b3_bc")

        # long-lived FFN-input tiles
        y1 = yp.tile([128, 4 * E], f32)
        y13 = y1[:].rearrange("p (c e) -> p c e", c=4)
        ylnT = yp.tile([128, EC * MYR], f16)
        ylnT3 = ylnT[:].rearrange("p (e s) -> p e s", e=EC)
        xmy_sb = yp.tile([128, 4 * E], f32)
        xmy3 = xmy_sb[:].rearrange("p (c e) -> p c e", c=4)

        def finish(J):
            # residual + LN2 for the 128 rows this core received
            rs_sb = yp.tile([128, E], f16, tag="rs_sb",
                            name=f"rs_sb_{J}", bufs=4)
            nc.sync.dma_start(rs_sb[:], rs_J[J][:, :])
            nc.vector.tensor_add(y13[:, J, :], rs_sb[:], xmy3[:, J, :])
            lny = lnyp.tile([128, E], f16, tag="lny")
            _layernorm_chunk(nc, lnstat, y13[:, J, :], lny[:])
            # b3 bypasses LN2: add into y1 after LN2 consumed it
            nc.vector.tensor_tensor(
                y13[:, J, :], y13[:, J, :], b3_bc[:], ALU.add
            )
            nc.sync.dma_start_transpose(
                ylnT3[:, :, J * 128:(J + 1) * 128], lny[:]
            )
        finish_fns = [finish]

        with tc.tile_pool(name="attnp", bufs=1) as attnp, \
             tc.tile_pool(name="xp", bufs=2) as xp, \
             tc.tile_pool(name="ln1Tp", bufs=2) as ln1Tp, \
             tc.tile_pool(name="lnxp", bufs=2) as lnxp, \
             tc.tile_pool(name="pexp", bufs=3) as pexp, \
             tc.tile_pool(name="projp", bufs=1) as projp, \
             tc.tile_pool(name="zp", bufs=2) as zp, \
             tc.tile_pool(name="psP", bufs=2, space="PSUM") as psP, \
             tc.tile_pool(name="psO", bufs=2, space="PSUM") as psO, \
             tc.tile_pool(name="psM", bufs=2, space="PSUM") as psM:

            def fetch_x(J):
                x_blk = xp.tile([128, 4 * E], f32, tag="xblk",
                                name=f"xblk_{J}")
                xp_tiles[J] = x_blk[:].rearrange("p (s e) -> p s e", s=4)
                nc.sync.dma_start(
                    xp_tiles[J][:, :, :], xb3[:, 4 * J:4 * (J + 1), :])

            fetch_x(0)
            nc.sync.dma_start(cf16[:], t["cf16"][:])
            nc.sync.dma_start(cf32[:], t["cf32"][:])
            fetch_x(1)
            nc.sync.dma_start(ones_row[:], t["ones_row"][:])
            nc.vector.memset(eshift[:], EXP_SHIFT)

            # Per-head q tensors with a 65th contraction row:
            # qa (key side): row 64 = key-mask bias / SCALE
            # qp (query side): row 64 = 1.0
            qa = [attnp.tile([65, S], f16, name=f"qa_{h}") for h in range(HPC)]
            qp = [attnp.tile([65, S], f16, name=f"qp_{h}") for h in range(HPC)]
            v_sb = attnp.tile([128, SC * VW], f16)
            v3 = v_sb[:].rearrange("p (s v) -> p s v", s=SC)
            oT01 = attnp.tile([128, S], f16)
            oT2 = attnp.tile([64, S], f16)

            ln_views = {}

            def ln1_qv(J, prefetch):
                # ---- LN1 + one DMA-crossbar transpose for token block J ----
                x3 = xp_tiles[J]
                lnx = lnxp.tile([128, 4 * E], f16, tag="lnx")
                for k in range(4):
                    _layernorm_chunk(nc, lnstat, x3[:, k, :],
                                     lnx[:, k * E:(k + 1) * E])
                ln1T_J = ln1Tp.tile([128, 4 * EC * 128], f16, tag="ln1T")
                nc.sync.dma_start_transpose(
                    ln1T_J[:].rearrange("p (b s) -> p b s", b=4 * EC), lnx[:])
                if prefetch is not None:
                    fetch_x(prefetch)
                # layout [p, k, e, s]: token kk*128+ss lives at [:, kk, :, ss]
                ln4 = ln1T_J[:].rearrange("p (k e s) -> p k e s", k=4, e=EC)
                ln_views[J] = ln4

                # ---- Q for block J (writes both qa and qp data rows) ----
                for g in range(2):
                    m = 128 if g == 0 else 64
                    pq = psM.tile([128, 512], f32, tag="pm")
                    for ec in range(EC):
                        nc.tensor.matmul(
                            pq[:m, :],
                            wq3[:, ec, g * 128:g * 128 + m],
                            ln4[:, :, ec, :],
                            start=(ec == 0), stop=(ec == EC - 1),
                        )
                    Jc = slice(J * 512, (J + 1) * 512)
                    if g == 0:
                        nc.vector.tensor_scalar_add(
                            qa[0][0:64, Jc], pq[0:64, :], bqc[0:64, 0:1])
                        nc.vector.tensor_scalar_add(
                            qp[0][0:64, Jc], pq[0:64, :], bqc[0:64, 0:1])
                        nc.vector.tensor_scalar_add(
                            qa[1][0:64, Jc], pq[64:128, :], bqc[64:128, 0:1])
                        nc.vector.tensor_scalar_add(
                            qp[1][0:64, Jc], pq[64:128, :], bqc[64:128, 0:1])
                    else:
                        nc.vector.tensor_scalar_add(
                            qa[2][0:64, Jc], pq[0:64, :], bqc[0:64, 1:2])
                        nc.vector.tensor_scalar_add(
                            qp[2][0:64, Jc], pq[0:64, :], bqc[0:64, 1:2])

                # ---- V for the 4 s-chunks of block J ----
                for k in range(4):
                    pv = psM.tile([128, VW], f32, tag="pm")
                    for ec in range(EC):
                        nc.tensor.matmul(
                            pv[:, :],
                            ln4[:, k, ec, :],
                            wv3[:, ec, :],
                            start=(ec == 0), stop=(ec == EC - 1),
                        )
                    nc.vector.tensor_tensor(
                        v3[:, 4 * J + k, :], pv[:, :], bv_bc[:], ALU.add
                    )

            def attn_wo_rs(J):
                # ---- attention for block J ----
                ntc = 4 * J + 4
                for hh in range(HPC):
                    oT = oT01 if hh < 2 else oT2
                    op_off = 64 * hh if hh < 2 else 0
                    po = psO.tile([128, 512], f32, tag="po")
                    for pr in range(ntc // 2):
                        tc0, tc1 = 2 * pr, 2 * pr + 1
                        k0, k1 = tc0 - 4 * J, tc1 - 4 * J
                        off0 = 128 * k0 if k0 > 0 else 0
                        off1 = 128 * k1 if k1 > 0 else 0
                        w0, w1_ = 512 - off0, 512 - off1
                        pp = psP.tile([128, 1024], f32, tag="pp")
                        nc.tensor.matmul(
                            pp[:, 0:w0],
                            qa[hh][:, tc0 * 128:(tc0 + 1) * 128],
                            qp[hh][:, J * 512 + off0:(J + 1) * 512],
                            start=True, stop=True,
                        )
                        nc.tensor.matmul(
                            pp[:, w0:w0 + w1_],
                            qa[hh][:, tc1 * 128:(tc1 + 1) * 128],
                            qp[hh][:, J * 512 + off1:(J + 1) * 512],
                            start=True, stop=True,
                        )
                        pe = pexp.tile([128, 1024], f16, tag="pe")
                        nc.scalar.activation(
                            pe[:, 0:w0 + w1_], pp[:, 0:w0 + w1_], AF.Exp,
                            bias=eshift[:], scale=SCALE,
                        )
                        if k0 >= 0:
                            nc.vector.tensor_mul(
                                pe[:, 0:128], pe[:, 0:128], m_band[:])
                            nc.vector.tensor_add(
                                pe[:, 0:128], pe[:, 0:128], dd3[:, 4 * J + k0, :])
                        if k1 >= 0:
                            nc.vector.tensor_mul(
                                pe[:, w0:w0 + 128], pe[:, w0:w0 + 128], m_band[:])
                            nc.vector.tensor_add(
                                pe[:, w0:w0 + 128], pe[:, w0:w0 + 128],
                                dd3[:, 4 * J + k1, :])
                        hs = slice(65 * hh, 65 * hh + 65)
                        nc.tensor.matmul(
                            po[:65, off0:512], v3[:, tc0, hs], pe[:, 0:w0],
                            start=(tc0 == 0), stop=False,
                        )
                        nc.tensor.matmul(
                            po[:65, off1:512], v3[:, tc1, hs], pe[:, w0:w0 + w1_],
                            start=False, stop=(tc1 == ntc - 1),
                        )
                    # softmax divide: oT[:, Jblock] = po[:64] * (1/po[64])
                    # (copy Z to partition 0 first: reciprocal_approx_fast
                    # mis-addresses inputs with a nonzero base partition)
                    zcp = zp.tile([1, 512], f32, tag="zcp")
                    nc.vector.tensor_copy(zcp[:], po[64:65, :])
                    zr = zp.tile([1, 512], f32, tag="zr")
                    nc.vector.reciprocal_approx_fast(zr[:], zcp[:])
                    zrech = zp.tile([1, 512], f16, tag="zrech")
                    nc.vector.tensor_copy(zrech[:], zr[:])
                    pb = psM.tile([64, 512], f32, tag="pm")
                    nc.tensor.matmul(
                        pb[:], ones_row[0:1, 0:64], zrech[:],
                        start=True, stop=True,
                    )
                    pb_sb = zp.tile([64, 512], f16, tag="pb_sb")
                    nc.scalar.copy(pb_sb[:], pb[:])
                    nc.vector.tensor_tensor(
                        oT[op_off:op_off + 64, J * 512:(J + 1) * 512],
                        po[0:64, :], pb_sb[:], ALU.mult,
                    )

                # ---- Wo projection for this row block + partial RS ----
                prj = projp.tile([128, 4 * E], f16, tag="prj")
                prj3 = prj[:].rearrange("p (s e) -> p s e", s=4)
                for sl in range(4):
                    st = 4 * J + sl
                    for hf in range(2):
                        pw = psM.tile([128, 384], f32, tag="pm")
                        nc.tensor.matmul(
                            pw[:],
                            oT01[:, st * 128:(st + 1) * 128],
                            wo0[:, hf * 384:(hf + 1) * 384],
                            start=True, stop=False,
                        )
                        nc.tensor.matmul(
                            pw[:],
                            oT2[:, st * 128:(st + 1) * 128],
                            wo1[:, hf * 384:(hf + 1) * 384],
                            start=False, stop=True,
                        )
                        nc.vector.tensor_tensor(
                            prj3[:, sl, hf * 384:(hf + 1) * 384], pw[:],
                            bo_bc[:, hf * 384:(hf + 1) * 384], ALU.add
                        )
                nc.sync.dma_start(
                    proj_J[J][:, :].rearrange("(s p) e -> p s e", s=4), prj3)
                nc.gpsimd.collective_compute(
                    "ReduceScatter",
                    ALU.add,
                    replica_groups=[[0, 1, 2, 3], [4, 5, 6, 7]],
                    ins=[proj_J[J][:, :].opt()],
                    outs=[rs_J[J][:, :].opt()],
                )

            # software pipeline, blocks processed [1,2,3,0]: the last
            # block's attention is the cheapest so its RS tail before the
            # FFN is short; finishes are deferred far past their RS issue.
            ln1_qv(0, prefetch=2)
            for h in range(HPC):
                nc.sync.dma_start(qa[h][64:65, :], t["mbrow"][:])
                nc.sync.dma_start(qp[h][64:65, :], t["ones_s"][:])
            ln1_qv(1, prefetch=3)
            nc.sync.dma_start(xmy_sb[:], t["xmy"][:])
            ln1_qv(2, prefetch=None)
            attn_wo_rs(1)
            ln1_qv(3, prefetch=None)
            attn_wo_rs(2)
            attn_wo_rs(3)
            finish_fns[0](1)
            finish_fns[0](2)
            attn_wo_rs(0)
            finish_fns[0](3)

        # ---------------- Phase E: FFN on my 512 rows ----------------------
        w13 = t["w1"][:].rearrange("p (e f) -> p e f", e=EC)
        w23 = t["w2"][:].rearrange("p (f g) -> p f g", f=FC)

        with tc.tile_pool(name="ffp", bufs=1) as ffp:
            h1T = ffp.tile([128, FC * MYR], f16)
            h13 = h1T[:].rearrange("p (f s) -> p f s", f=FC)
            h2T = ffp.tile([128, FC * MYR], f16)
            h23 = h2T[:].rearrange("p (f s) -> p f s", f=FC)
            w3s = ffp.tile([128, FC * E], f16)
            w3s3 = w3s[:].rearrange("p (f e) -> p f e", f=FC)

            with tc.tile_pool(name="psF1", bufs=2, space="PSUM") as psF1, \
                 tc.tile_pool(name="psF2", bufs=4, space="PSUM") as psF2:
                # h1T = relu(W1^T @ yln^T + b1), in two phases: token
                # blocks 1-3 first (their ylnT cols land before the last
                # reduce-scatter), then block 0 (w1 tiles re-streamed)
                def w1_phase(phase, lo, hi):
                    for fc2 in range(FC // 2):
                        w1t = w1p.tile([128, EC * 256], f16, tag="w1t")
                        w1t3 = w1t[:].rearrange("p (e f) -> p e f", e=EC)
                        nc.sync.dma_start(
                            w1t3, w13[:, :, fc2 * 256:(fc2 + 1) * 256])
                        for j in range(2):
                            fc = 2 * fc2 + j
                            pf = psF1.tile([128, hi - lo], f32,
                                           tag=f"pf{phase}")
                            for ec in range(EC):
                                nc.tensor.matmul(
                                    pf[:],
                                    w1t3[:, ec, j * 128:(j + 1) * 128],
                                    ylnT3[:, ec, lo:hi],
                                    start=(ec == 0), stop=(ec == EC - 1),
                                )
                            nc.scalar.activation(
                                h13[:, fc, lo:hi], pf[:], AF.Relu,
                                bias=b1c[:, fc:fc + 1], scale=1.0,
                            )

                w1_phase(0, 128, 512)      # overlaps the last reduce-scatter
                finish_fns[0](0)
                w1_phase(1, 0, 128)

                # h2T = relu(W2^T @ h1T + b2); W3 stage prefetches alongside
                nc.scalar.dma_start(w3s3, t["w3"][:].rearrange(
                    "p (f e) -> p f e", f=FC))
                for f2b in range(6):
                    ph2 = [
                        psF2.tile([128, 512], f32, tag="ph2", name=f"ph2_{f2b}_{k}")
                        for k in range(4)
                    ]
                    for f1c4 in range(FC // 4):
                        w2t = w2p.tile([128, 4 * 512], f16, tag="w2t")
                        w2t3 = w2t[:].rearrange("p (j g) -> p j g", j=4)
                        nc.sync.dma_start(
                            w2t3,
                            w23[:, 4 * f1c4:4 * f1c4 + 4,
                                f2b * 512:(f2b + 1) * 512],
                        )
                        for j in range(4):
                            f1c = 4 * f1c4 + j
                            for k in range(4):
                                nc.tensor.matmul(
                                    ph2[k][:],
                                    w2t3[:, j, k * 128:(k + 1) * 128],
                                    h13[:, f1c, :],
                                    start=(f1c == 0), stop=(f1c == FC - 1),
                                )
                    for k in range(4):
                        fc2 = f2b * 4 + k
                        nc.scalar.activation(
                            h23[:, fc2, :], ph2[k][:], AF.Relu,
                            bias=b2c[:, fc2:fc2 + 1], scale=1.0,
                        )

            # out = h2 @ W3 + b3 + y1, st-outer so each 768-col block of the
            # output stores while the next block's matmuls run
            with tc.tile_pool(name="outp", bufs=1) as outp, \
                 tc.tile_pool(name="psF3", bufs=4, space="PSUM") as psF3:
                out_sb = outp.tile([128, 4 * E], f32)
                out3 = out_sb[:].rearrange("p (c e) -> p c e", c=4)
                out_d3 = t["out"][:].rearrange("p (c e) -> p c e", c=4)
                for st in range(4):
                    p3 = [psF3.tile([128, 384], f32, tag="p3",
                                    name=f"p3_{st}_{hf}")
                          for hf in range(2)]
                    for fc in range(FC):
                        for hf in range(2):
                            nc.tensor.matmul(
                                p3[hf][:],
                                h23[:, fc, st * 128:(st + 1) * 128],
                                w3s3[:, fc, hf * 384:(hf + 1) * 384],
                                start=(fc == 0), stop=(fc == FC - 1),
                            )
                    for hf in range(2):
                        nc.vector.tensor_add(
                            out3[:, st, hf * 384:(hf + 1) * 384],
                            p3[hf][:],
                            y13[:, st, hf * 384:(hf + 1) * 384],
                        )
                    nc.sync.dma_start(out_d3[:, st, :], out3[:, st, :])

def _build():
    if "nc" in _CACHE:
        return _CACHE["nc"]
    nc = bacc.Bacc("TRN2", target_bir_lowering=False, debug=False,
                   num_devices=NCORES)
    t = _declare_io(nc)
    with tile.TileContext(nc) as tc:
        _build_body(tc, t)
    nc.compile()
    _CACHE["nc"] = nc
    return nc


def _chunk_rows(a, p=128):
    """[N, M] -> [p, N//p, M] -> [p, (N//p)*M] row-chunk packing."""
    n, m = a.shape
    return np.ascontiguousarray(
        a.reshape(n // p, p, m).transpose(1, 0, 2).reshape(p, -1)
    )


def _prep_in_maps(inputs):
    x = np.asarray(inputs["x"], np.float32)
    Wq = np.asarray(inputs["Wq"], np.float32)
    bq = np.asarray(inputs["bq"], np.float32)
    Wv = np.asarray(inputs["Wv"], np.float32)
    bv = np.asarray(inputs["bv"], np.float32)
    Wo = np.asarray(inputs["Wo"], np.float32)
    bo = np.asarray(inputs["bo"], np.float32)
    ln1_g = np.asarray(inputs["ln1_g"], np.float32)
    ln1_b = np.asarray(inputs["ln1_b"], np.float32)
    W1 = np.asarray(inputs["W1"], np.float32)
    b1 = np.asarray(inputs["b1"], np.float32)
    W2 = np.asarray(inputs["W2"], np.float32)
    b2 = np.asarray(inputs["b2"], np.float32)
    W3 = np.asarray(inputs["W3"], np.float32)
    b3 = np.asarray(inputs["b3"], np.float32)
    ln2_g = np.asarray(inputs["ln2_g"], np.float32)
    ln2_b = np.asarray(inputs["ln2_b"], np.float32)
    mask = np.asarray(inputs["input_mask"])

    # Fold LN affine params into the following projections (exact algebra).
    Wq_eff = Wq * ln1_g[None, :, None]
    bq_eff = bq + np.einsum("e,hed->hd", ln1_b, Wq)
    Wv_eff = Wv * ln1_g[None, :, None]
    bv_eff = bv + np.einsum("e,hed->hd", ln1_b, Wv)
    W1_eff = W1 * ln2_g[:, None]
    b1_eff = b1 + ln2_b @ W1

    w1_p = _chunk_rows(W1_eff)                      # [128, 6*3072]
    b1c = np.ascontiguousarray(b1_eff.reshape(FC, 128).T)
    w2_p = _chunk_rows(W2)                          # [128, 24*3072]
    b2c = np.ascontiguousarray(b2.reshape(FC, 128).T)
    w3_p = _chunk_rows(W3)                          # [128, 24*768]
    b3_bc = np.broadcast_to(b3.astype(np.float32), (128, E)).copy()
    ones_row = np.ones((1, 512), np.float32)
    ones_s = np.ones((1, S), np.float32)

    # causal 0/1 band mask for the diagonal 128x128 band of each diag chunk
    tl = np.arange(128)[:, None]
    cl = np.arange(128)[None, :]
    m_band = (cl >= tl).astype(np.float16)

    f16c = np.float16
    in_maps = []
    for c in range(NCORES):
        b, r = c // R, c % R
        hs = [HPC * r + i for i in range(HPC)]

        xb = _chunk_rows(x[b])                      # [128, 16*768]
        myrows = np.concatenate(
            [np.arange(512 * J + 128 * r, 512 * J + 128 * r + 128)
             for J in range(4)]
        )
        xmy = _chunk_rows(x[b, myrows])

        Wq_my = np.concatenate([Wq_eff[h] for h in hs], axis=1)   # [E, 192]
        bq_my = np.concatenate([bq_eff[h] for h in hs])           # [192]
        wq_p = _chunk_rows(Wq_my)
        bqc = np.zeros((128, 2), np.float32)
        bqc[:, 0] = bq_my[:128]
        bqc[:64, 1] = bq_my[128:]

        Wv_aug = np.zeros((E, VW), np.float32)
        bv1 = np.zeros((1, VW), np.float32)
        for i, h in enumerate(hs):
            Wv_aug[:, 65 * i: 65 * i + 64] = Wv_eff[h]
            bv1[0, 65 * i: 65 * i + 64] = bv_eff[h]
            bv1[0, 65 * i + 64] = 1.0
        bv_bc = np.broadcast_to(bv1, (128, VW)).copy()
        wv_p = _chunk_rows(Wv_aug)

        wo0 = np.ascontiguousarray(Wo[hs[0] * D: hs[0] * D + 128])
        wo1 = np.zeros((128, E), np.float32)
        wo1[0:64] = Wo[hs[2] * D: hs[2] * D + 64]
        bo_bc = np.broadcast_to(
            (bo if r == 0 else np.zeros_like(bo)).astype(np.float32), (128, E)
        ).copy()

        # key-mask bias row, pre-divided by the exp scale (applied via the
        # augmented contraction row in the score matmul)
        mbrow = (np.where(mask[b] == 0, MASK_BIAS / SCALE, 0.0)
                 .astype(np.float32)[None, :])

        bad = (np.cumsum(mask[b]) == 0).astype(np.float32) * \
            np.float32(np.exp(EXP_SHIFT))
        dd_b = np.zeros((128, 16, 128), np.float16)
        i128 = np.arange(128)
        for jk in range(16):
            dd_b[i128, jk, i128] = bad[jk * 128 + i128]

        cf16 = np.zeros((128, C16N), np.float16)
        for name, arr in [("wq", wq_p), ("wv", wv_p), ("wo0", wo0),
                          ("wo1", wo1), ("m_band", m_band),
                          ("dd_band", dd_b.reshape(128, -1))]:
            o, n = C16[name]
            cf16[:, o:o + n] = arr.astype(f16c)
        cf32 = np.zeros((128, C32N), np.float32)
        for name, arr in [("bqc", bqc), ("bv_bc", bv_bc), ("bo_bc", bo_bc),
                          ("b1c", b1c), ("b2c", b2c), ("b3_bc", b3_bc)]:
            o, n = C32[name]
            cf32[:, o:o + n] = arr

        in_maps.append({
            "xb": xb, "xmy": xmy,
            "cf16": cf16, "cf32": cf32,
            "mbrow": mbrow.astype(f16c),
            "ones_s": ones_s.astype(f16c),
            "ones_row": ones_row.astype(f16c),
            "w1": w1_p.astype(f16c),
            "w2": w2_p.astype(f16c),
            "w3": w3_p.astype(f16c),
        })
    return in_maps


def _gather(results):
    y = np.empty((B, S, E), np.float32)
    for c in range(NCORES):
        b, r = c // R, c % R
        o = results[c]["out"].reshape(128, 4, E).transpose(1, 0, 2).reshape(MYR, E)
        myrows = np.concatenate(
            [np.arange(512 * J + 128 * r, 512 * J + 128 * r + 128)
             for J in range(4)]
        )
        y[b, myrows] = o
    return y


def run(inputs, **spmd_kwargs):
    nc = _build()
    in_maps = _prep_in_maps(inputs)
    res = run_bass_kernel_spmd(nc, in_maps, core_ids=list(range(NCORES)),
                               **spmd_kwargs)
    return _gather(res.results), res


def kernel(**inputs) -> np.ndarray:
    y, _ = run(inputs)
    return y
